# revision 1
# baseline (speedup 1.0000x reference)
"""Causal self-attention (B=2, T=2048, C=1024, H=16) on 8 trn2 NeuronCores.

Sharding: batch x head-group. Core c handles batch b = c//4 and heads
[4*(c%4), 4*(c%4)+4). Each core computes qkv for its head slice, causal
attention, and a partial c_proj ([T, C] over its 256 input rows of W_proj);
the host sums the 4 partials per batch (data-parallel over b, tensor-parallel
over heads with the all-reduce done on host).  ~232us measured on HW,
rel err ~3.5e-3 (bf16 operands; fp32 accumulate everywhere).

Device dataflow (per core):
  - qT, kT computed in [D', T] layout (D' = 256 local head dims), v in [T, D']
    layout, all from host-pre-transposed bf16 xT [C, T].
  - attention per head: S^T[k, q] = kT.T-slice @ qT-slice so that softmax's
    key dim is the PSUM partition dim; the padding mask becomes a
    per-partition bias of the exp activation.  exp(S/8 + maskNEG) -> U^T.
    Causal masking = one [128,128] upper-tri elementwise multiply per
    diagonal tile.  O^T[d, q] accumulates lhsT=v_aug [k, 97] (cols 64:96
    zero, col 96 = 1.0, so psum row 96 collects the softmax denominator for
    free), rhs=U^T.
  - normalize: denominator [1,1024] -> DMA-reshape to [64,16] -> reciprocal
    -> DMA broadcast [64,1024] -> multiply = yT [256, T], exactly the lhsT
    of c_proj.  partial = yT.T @ W_proj[256 rows].  Host adds b_proj.

Schedule notes (PE HAM-clock discipline: the PE clock halves after idle
windows and re-warms only under sustained dense work):
  - a dependency-free pre-warm matmul burst runs at t=0, sized to cover the
    input-DMA window, so real work starts at full clock;
  - qkv is emitted chunk-outer so PE work arrives in lockstep with its DMAs;
  - qkv t>=1024 tblocks are braided between attention q<1024 heads, and
    c_proj t<1024 tiles between attention q>=1024 heads, so dense K=128
    matmul bursts keep the array warm through the exp-paced attention;
  - attention emits S(j+1) before O(j) (engine instruction order is static);
  - heads run 2,3,0,1 and c_proj accumulates W_proj chunk 1 before chunk 0,
    the first tiles of the final proj batch staged through SBUF, so the last
    heads' normalize chains never idle the PE;
  - bulk output DMAs ride the gpsimd queue; the latency-critical normalize
    chain DMAs ride the otherwise-empty Sync queue.
"""

import contextlib
import functools
import sys

sys.path.insert(0, "/opt/trn_rl_repo")

import numpy as np

import concourse.bacc as bacc
import concourse.mybir as mybir
import concourse.tile as tile
from concourse import bass_utils
from concourse.alu_op_type import AluOpType

B, T, C, H, D = 2, 2048, 1024, 16, 64
NEG = -1e10
NCORES = 8
HEADS_PER_CORE = 4
DLOC = HEADS_PER_CORE * D  # 256 local head dims per core
F32 = mybir.dt.float32
F32R = mybir.dt.float32r
BF16 = mybir.dt.bfloat16
AF = mybir.ActivationFunctionType

# bf16 for the qkv input matmuls (x, W_attn slices) and the c_proj matmul
# (yT, W_proj): halves the input-DMA window that gates the kernel start.
# S^T / O^T attention matmuls stay fp32r.
USE_BF16_INPUTS = True
IN_DT = BF16 if USE_BF16_INPUTS else F32R
# ~duration of junk pre-warm matmuls covering the input-DMA window (ns)
WARM_NS = 21000
# junk matmuls per attention j-step: dependency-free PE-array activity that
# fills the pipeline's micro-gaps so the HAM clock never re-throttles
N_FILL = 0

NTB = T // 512  # 4 t-blocks in qkv phase
NKC = T // 128  # 16 k-chunks
NQB = 2  # attention q-blocks of 1024


def _r(ap):
    return ap.bitcast(F32R)


def _pieces(a, end=1024):
    """Split [a, end) at 512-boundaries (psum bank boundaries)."""
    cuts = [a]
    b = (a // 512 + 1) * 512
    while b < end:
        cuts.append(b)
        b += 512
    cuts.append(end)
    return list(zip(cuts[:-1], cuts[1:]))


class Ctx:
    pass


def _emit_prewarm(nc, g):
    """Dependency-free fp32 matmuls sized to cover the initial input-DMA
    window: the PE warms up on junk (instead of idling cold) and hands off
    at full clock to a fully-fed dense qkv stream.  Cold-busy is what the
    HAM punishes; cold-idle under DMA is free, but a ragged DMA-paced start
    re-throttles the clock over and over."""
    # fp32 N=512 matmul = 4 cyc/col: ~1.7us cold, ~0.85us warm; warm-up
    # transition after ~3.4us
    n = 3 + max(0, int((WARM_NS - 5200) / 880))
    ps = g.pool_x.tile([128, 1024], F32, tag="px", name="warm_ps")
    for i in range(n):
        nc.tensor.matmul(
            ps[:, 0:512],
            g.warm_sb[:, 0:128],
            g.warm_sb,
            start=(i == 0),
            stop=(i == n - 1),
        )
    wsink = g.rnpool.tile([1, 128], F32, tag="wsink", name="wsink")
    nc.vector.tensor_copy(wsink, ps[0:1, 0:128])
    nc.sync.dma_start(out=g.rn_dram.ap()[0:1, 0:128], in_=wsink)


def _emit_qkv_tblock(nc, g, tb, with_weights=False):
    """qkv projections for t in [tb*512, (tb+1)*512), chunk-outer: all eight
    chains step to chunk cc together, right after chunk cc's DMAs land."""
    xts = [
        g.xpool.tile([128, 512], IN_DT, tag=f"xts{cc}", name=f"xts{cc}")
        for cc in range(8)
    ]
    psq = g.pool_x.tile([128, 1024], F32, tag="px", name="psq")
    psk = g.pool_x.tile([128, 1024], F32, tag="px", name="psk")
    # four concurrent v chains need four distinct psum banks (an accumulation
    # group's start=True zeroes its whole 2KB bank): chain ts lives in tile
    # ts//2 at column offset (ts%2)*512
    psv = [
        g.pool_o.tile([128, 1024], F32, tag="pso", name="psvA"),
        g.pool_o.tile([128, 1024], F32, tag="pso", name="psvB"),
    ]

    def vslice(ts, width=256):
        return psv[ts // 2][:, (ts % 2) * 512 : (ts % 2) * 512 + width]
    for cc in range(8):
        nc.sync.dma_start(out=xts[cc], in_=g.xT_r[:, cc, tb * 512 : (tb + 1) * 512])
        if with_weights:
            for wsb, src in ((g.wq_sb, g.wq_src), (g.wk_sb, g.wk_src), (g.wv_sb, g.wv_src)):
                nc.sync.dma_start(out=wsb[cc], in_=src[:, cc, :])
        st, sp = cc == 0, cc == 7
        for dt_ in range(2):
            nc.tensor.matmul(
                psq[:, dt_ * 512 : (dt_ + 1) * 512],
                g.wq_sb[cc][:, dt_ * 128 : (dt_ + 1) * 128],
                xts[cc],
                start=st,
                stop=sp,
            )
            nc.tensor.matmul(
                psk[:, dt_ * 512 : (dt_ + 1) * 512],
                g.wk_sb[cc][:, dt_ * 128 : (dt_ + 1) * 128],
                xts[cc],
                start=st,
                stop=sp,
            )
        for ts in range(4):
            nc.tensor.matmul(
                vslice(ts),
                xts[cc][:, ts * 128 : (ts + 1) * 128],
                g.wv_sb[cc],
                start=st,
                stop=sp,
            )
    for dt_ in range(2):
        nc.vector.tensor_scalar(
            out=g.qT_sb[:, dt_, tb * 512 : (tb + 1) * 512],
            in0=psq[:, dt_ * 512 : (dt_ + 1) * 512],
            scalar1=g.bq_sb[:, dt_ : dt_ + 1],
            scalar2=None,
            op0=AluOpType.add,
        )
        nc.vector.tensor_scalar(
            out=g.kT_sb[:, dt_, tb * 512 : (tb + 1) * 512],
            in0=psk[:, dt_ * 512 : (dt_ + 1) * 512],
            scalar1=g.bk_sb[:, dt_ : dt_ + 1],
            scalar2=None,
            op0=AluOpType.add,
        )
    for ts in range(4):
        kc = tb * 4 + ts
        for h in range(4):
            nc.vector.tensor_tensor(
                out=g.vaug[h][:, kc, 0:D],
                in0=vslice(ts)[:, h * D : (h + 1) * D],
                in1=g.bvb_sb[:, h * D : (h + 1) * D],
                op=AluOpType.add,
            )


def _emit_attention_block(nc, g, h, m):
    """One head x one 1024-wide q-block of causal attention."""
    prow = (h % 2) * 64
    pi = h // 2
    njs = 8 * m + 8
    pso = g.pool_o.tile([128, 1024], F32, tag="pso", name="pso")
    last_bank0 = 8 * m + 3
    uts = {}

    def emit_S_exp(j):
        # S^T then exp; the O^T consuming exp(j) is emitted after S(j+1) so
        # the PE's static instruction order never waits on the ACT engine
        a = max(0, 128 * j - 1024 * m)
        pss = g.pool_x.tile([128, 1024], F32, tag="px", name="pss")
        for c0, c1 in _pieces(a):
            nc.tensor.matmul(
                pss[:, c0:c1],
                g.kT_sb[prow : prow + 64, pi, j * 128 : (j + 1) * 128],
                g.qT_sb[prow : prow + 64, pi, m * 1024 + c0 : m * 1024 + c1],
                start=True,
                stop=True,
            )
        ut = g.utpool.tile([128, 1024], BF16, tag="ut", name="ut")
        uts[j] = ut
        nc.scalar.activation(
            out=ut[:, a:1024],
            in_=pss[:, a:1024],
            func=AF.Exp,
            bias=g.mneg_sb[:, j : j + 1],
            scale=0.125,
        )
        if j >= 8 * m:
            nc.vector.tensor_mul(ut[:, a : a + 128], ut[:, a : a + 128], g.tri_sb)

    def emit_O(j):
        a = max(0, 128 * j - 1024 * m)
        ut = uts.pop(j)
        for c0, c1 in _pieces(a):
            stop = j == (last_bank0 if c0 < 512 else njs - 1)
            nc.tensor.matmul(
                pso[0:97, c0:c1],
                g.vaug[h][:, j, :],
                ut[:, c0:c1],
                start=(j == 0),
                stop=stop,
            )

    def emit_norm():
        # normalize: yT[h rows, m block] = O^T * (1/denom) broadcast.  The
        # denominator row is [1, 1024]; reciprocal there runs on one DVE
        # lane (6.5us), so DMA-reshape it to [64, 16] first.  All chain DMAs
        # ride the Sync queue, which carries no bulk traffic by this point
        # (outputs drain via the gpsimd queue).
        hm = h * NQB + m
        dn = g.rnpool.tile([1, 1024], F32, tag="dn", name="dn")
        nc.scalar.copy(dn, pso[96:97, :])
        nc.sync.dma_start(out=g.rn_dram.ap()[hm : hm + 1, :], in_=dn)
        dn_rs = g.rnpool.tile([64, 16], F32, tag="dn_rs", name="dn_rs")
        nc.sync.dma_start(
            out=dn_rs, in_=g.rn_dram.ap()[hm, :].rearrange("(p f) -> p f", p=64)
        )
        rr = g.rnpool.tile([64, 16], F32, tag="rr", name="rr")
        nc.vector.reciprocal(rr, dn_rs)
        nc.sync.dma_start(
            out=g.rn2_dram.ap()[hm, :].rearrange("(p f) -> p f", p=64), in_=rr
        )
        rnb = g.rnpool.tile([64, 1024], F32, tag="rnb", name="rnb")
        nc.sync.dma_start(
            out=rnb, in_=g.rn2_dram.ap()[hm : hm + 1, :].partition_broadcast(64)
        )
        nc.vector.tensor_tensor(
            out=g.yT_sb[prow : prow + 64, pi, m * 1024 : (m + 1) * 1024],
            in0=pso[0:D, :],
            in1=rnb,
            op=AluOpType.mult,
        )

    emit_S_exp(0)
    for j in range(1, njs):
        emit_S_exp(j)
        emit_O(j - 1)
    emit_O(njs - 1)
    emit_norm()


def _emit_proj_tile(nc, g, i, out):
    """One plain [128, C] c_proj tile (both chunks ready)."""
    psp = g.pool_x.tile([128, 1024], F32, tag="px", name="psp")
    for step, ic in enumerate((1, 0)):
        for c0, c1 in _pieces(0):
            nc.tensor.matmul(
                psp[:, c0:c1],
                g.yT_sb[:, ic, i * 128 : (i + 1) * 128],
                g.wp_sb[:, ic, c0:c1],
                start=(step == 0),
                stop=(step == 1),
            )
    ob = g.outp.tile([128, C], F32, tag="ob_plain", name="ob_plain", bufs=3)
    if i % 2 == 0:
        nc.vector.tensor_copy(ob, psp)
    else:
        nc.scalar.copy(ob, psp)
    nc.gpsimd.dma_start(out=out.ap()[i * 128 : (i + 1) * 128, :], in_=ob)


def _emit_proj_batch(nc, g, istart, out, n_staged=4):
    """Eight [128, C] c_proj output tiles.  Chunk ic=1 (heads 2,3, which
    finish first) accumulates before ic=0.  The first n_staged tiles run
    their ic=1 matmul groups back-to-back (results staged through SBUF) so
    the PE has work while the last heads' normalize chains drain; their ic=0
    groups follow, joined by a DVE add."""
    obs = {}
    for i in range(istart, istart + n_staged):
        ob = g.outp.tile(
            [128, C], F32, tag=f"ob{i - istart}", name=f"ob{i - istart}", bufs=1
        )
        obs[i] = ob
        ps1 = g.pool_x.tile([128, 1024], F32, tag="px", name="ps1")
        for c0, c1 in _pieces(0):
            nc.tensor.matmul(
                ps1[:, c0:c1],
                g.yT_sb[:, 1, i * 128 : (i + 1) * 128],
                g.wp_sb[:, 1, c0:c1],
                start=True,
                stop=True,
            )
        nc.vector.tensor_copy(ob, ps1)
    for i in range(istart, istart + n_staged):
        ps0 = g.pool_x.tile([128, 1024], F32, tag="px", name="ps0")
        for c0, c1 in _pieces(0):
            nc.tensor.matmul(
                ps0[:, c0:c1],
                g.yT_sb[:, 0, i * 128 : (i + 1) * 128],
                g.wp_sb[:, 0, c0:c1],
                start=True,
                stop=True,
            )
        nc.vector.tensor_tensor(out=obs[i], in0=ps0, in1=obs[i], op=AluOpType.add)
        nc.gpsimd.dma_start(out=out.ap()[i * 128 : (i + 1) * 128, :], in_=obs[i])
    for i in range(istart + n_staged, istart + 8):
        psp = g.pool_x.tile([128, 1024], F32, tag="px", name="psp")
        for step, ic in enumerate((1, 0)):
            for c0, c1 in _pieces(0):
                nc.tensor.matmul(
                    psp[:, c0:c1],
                    g.yT_sb[:, ic, i * 128 : (i + 1) * 128],
                    g.wp_sb[:, ic, c0:c1],
                    start=(step == 0),
                    stop=(step == 1),
                )
        ob = g.outp.tile([128, C], F32, tag="ob_plain", name="ob_plain", bufs=3)
        if i % 2 == 0:
            nc.vector.tensor_copy(ob, psp)
        else:
            nc.scalar.copy(ob, psp)
        nc.gpsimd.dma_start(out=out.ap()[i * 128 : (i + 1) * 128, :], in_=ob)


def _build(ctx, nc, tc, ins, out, rn_dram, rn2_dram):
    g = Ctx()
    g.rn_dram, g.rn2_dram = rn_dram, rn2_dram

    singles = ctx.enter_context(tc.tile_pool(name="singles", bufs=1))
    g.pool_x = ctx.enter_context(tc.tile_pool(name="pool_x", bufs=2, space="PSUM"))
    g.pool_o = ctx.enter_context(tc.tile_pool(name="pool_o", bufs=2, space="PSUM"))
    g.xpool = ctx.enter_context(tc.tile_pool(name="xpool", bufs=2))
    g.utpool = ctx.enter_context(tc.tile_pool(name="utpool", bufs=6))
    g.rnpool = ctx.enter_context(tc.tile_pool(name="rnpool", bufs=2))
    g.outp = ctx.enter_context(tc.tile_pool(name="outp", bufs=1))

    # tri mask first: the pre-warm burst depends only on it
    g.tri_sb = singles.tile([128, 128], BF16, name="tri_sb")
    nc.sync.dma_start(out=g.tri_sb, in_=ins["tri"].ap())
    g.warm_sb = singles.tile([128, 512], F32, name="warm_sb")
    nc.vector.memset(g.warm_sb, 0.5)
    g.warmb_sb = singles.tile([128, 256], BF16, name="warmb_sb")
    nc.vector.tensor_copy(g.warmb_sb, g.warm_sb[:, 0:256])
    _emit_prewarm(nc, g)

    # --- resident weights / constants (matmul inputs are F32R) ----------
    # per-chunk weight tiles, DMA'd interleaved with the first x chunks
    g.wq_sb = [singles.tile([128, DLOC], IN_DT, name=f"wq{c}") for c in range(8)]
    g.wk_sb = [singles.tile([128, DLOC], IN_DT, name=f"wk{c}") for c in range(8)]
    g.wv_sb = [singles.tile([128, DLOC], IN_DT, name=f"wv{c}") for c in range(8)]
    g.wq_src = ins["wq"].ap().rearrange("(c p) m -> p c m", p=128)
    g.wk_src = ins["wk"].ap().rearrange("(c p) m -> p c m", p=128)
    g.wv_src = ins["wv"].ap().rearrange("(c p) m -> p c m", p=128)

    g.bq_sb = singles.tile([128, 2], F32, name="bq_sb")
    g.bk_sb = singles.tile([128, 2], F32, name="bk_sb")
    nc.sync.dma_start(out=g.bq_sb, in_=ins["bq"].ap().rearrange("i p -> p i"))
    nc.sync.dma_start(out=g.bk_sb, in_=ins["bk"].ap().rearrange("i p -> p i"))
    g.bvb_sb = singles.tile([128, DLOC], F32, name="bvb_sb")
    nc.sync.dma_start(out=g.bvb_sb, in_=ins["bv"].ap().partition_broadcast(128))
    g.mneg_sb = singles.tile([128, NKC], F32, name="mneg_sb")
    nc.sync.dma_start(out=g.mneg_sb, in_=ins["mneg"].ap())

    ones16 = singles.tile([128, NKC], F32, name="ones16")
    nc.vector.memset(ones16, 1.0)

    # --- persistent activations -----------------------------------------
    g.qT_sb = singles.tile([128, 2, T], BF16, tag="qT", name="qT_sb")
    g.kT_sb = singles.tile([128, 2, T], BF16, tag="kT", name="kT_sb")
    g.vaug = [
        singles.tile([128, NKC, 97], BF16, tag=f"vaug{h}", name=f"vaug{h}")
        for h in range(4)
    ]
    zpad = singles.tile([128, NKC * 32], F32, name="zpad")
    nc.vector.memset(zpad, 0.0)
    for h in range(4):
        # cols 64:96 zero, col 96 = 1.0: the O^T matmul then emits the
        # softmax denominator in psum row 96 (engine partition reads must be
        # 32-aligned), leaving rows [64,96) for junk warm-keeper matmuls
        nc.gpsimd.tensor_copy(
            g.vaug[h][:, :, D:96], zpad.rearrange("p (a b) -> p a b", a=NKC)
        )
        nc.gpsimd.tensor_copy(g.vaug[h][:, :, 96], ones16)
    g.yT_sb = singles.tile([128, 2, T], IN_DT, tag="yT", name="yT_sb")

    g.xT_r = ins["xT"].ap().rearrange("(c p) t -> p c t", p=128)

    # qkv for t < 1024
    _emit_qkv_tblock(nc, g, 0, with_weights=True)
    _emit_qkv_tblock(nc, g, 1)
    # attention for q < 1024 with qkv t>=1024 braided between heads: the
    # dense K=128 qkv chains keep the PE array warm through the exp-paced
    # attention stream.  Heads 2,3 first so c_proj (chunk-1-first) can start
    # before heads 0,1 finish.
    _emit_attention_block(nc, g, 2, 0)
    _emit_qkv_tblock(nc, g, 2)
    _emit_attention_block(nc, g, 3, 0)
    _emit_qkv_tblock(nc, g, 3)
    _emit_attention_block(nc, g, 0, 0)
    # c_proj weights (needed from proj phase on; DMA fits mid-kernel)
    g.wp_sb = singles.tile([128, 2, C], IN_DT, name="wp_sb")
    wp_src = ins["wproj"].ap().rearrange("(i p) n -> p i n", p=128)
    for ic in range(2):
        nc.sync.dma_start(out=g.wp_sb[:, ic, :], in_=wp_src[:, ic, :])
    _emit_attention_block(nc, g, 1, 0)
    # attention for q >= 1024 with proj t<1024 tiles braided between heads
    _emit_attention_block(nc, g, 2, 1)
    for i in (0, 1, 2):
        _emit_proj_tile(nc, g, i, out)
    _emit_attention_block(nc, g, 3, 1)
    for i in (3, 4, 5):
        _emit_proj_tile(nc, g, i, out)
    _emit_attention_block(nc, g, 0, 1)
    for i in (6, 7):
        _emit_proj_tile(nc, g, i, out)
    _emit_attention_block(nc, g, 1, 1)
    # proj for t >= 1024 (first tiles staged to bridge the normalize chains)
    _emit_proj_batch(nc, g, 8, out)


@functools.lru_cache(maxsize=1)
def _program():
    nc = bacc.Bacc("TRN2", target_bir_lowering=False, debug=False)
    shapes = {
        "xT": ([C, T], IN_DT),
        "wq": ([C, DLOC], IN_DT),
        "wk": ([C, DLOC], IN_DT),
        "wv": ([C, DLOC], IN_DT),
        "bq": ([2, 128], F32),
        "bk": ([2, 128], F32),
        "bv": ([1, DLOC], F32),
        "wproj": ([DLOC, C], IN_DT),
        "mneg": ([128, NKC], F32),
        "tri": ([128, 128], BF16),
    }
    ins = {
        name: nc.dram_tensor(name, shape, dt_, kind="ExternalInput")
        for name, (shape, dt_) in shapes.items()
    }
    out = nc.dram_tensor("out", [T, C], F32, kind="ExternalOutput")
    rn_dram = nc.dram_tensor("rn_scratch", [8, 1024], F32, kind="Internal")
    rn2_dram = nc.dram_tensor("rn2_scratch", [8, 1024], F32, kind="Internal")
    with tile.TileContext(nc) as tc, contextlib.ExitStack() as ctx:
        _build(ctx, nc, tc, ins, out, rn_dram, rn2_dram)
    nc.compile()
    return nc


def make_in_maps(x, attention_mask, W_attn, b_attn, W_proj, b_proj):
    import ml_dtypes

    in_np = ml_dtypes.bfloat16 if USE_BF16_INPUTS else np.float32
    x = np.ascontiguousarray(np.asarray(x, dtype=np.float32))
    attention_mask = np.asarray(attention_mask, dtype=np.float32)
    W_attn = np.asarray(W_attn, dtype=np.float32)
    b_attn = np.asarray(b_attn, dtype=np.float32)
    W_proj = np.asarray(W_proj, dtype=np.float32)

    tri = (np.arange(128)[None, :] >= np.arange(128)[:, None]).astype(np.float32)
    in_maps = []
    for c in range(NCORES):
        b = c // 4
        g = c % 4
        cols = slice(g * DLOC, (g + 1) * DLOC)
        xT = np.ascontiguousarray(x[b].T.astype(in_np))
        mneg = np.ascontiguousarray((attention_mask[b] * NEG).reshape(NKC, 128).T)
        in_maps.append(
            {
                "xT": xT,
                "wq": np.ascontiguousarray(W_attn[:, cols].astype(in_np)),
                "wk": np.ascontiguousarray(W_attn[:, C : 2 * C][:, cols].astype(in_np)),
                "wv": np.ascontiguousarray(
                    W_attn[:, 2 * C : 3 * C][:, cols].astype(in_np)
                ),
                "bq": np.ascontiguousarray(b_attn[cols].reshape(2, 128)),
                "bk": np.ascontiguousarray(b_attn[C : 2 * C][cols].reshape(2, 128)),
                "bv": np.ascontiguousarray(b_attn[2 * C : 3 * C][cols].reshape(1, DLOC)),
                "wproj": np.ascontiguousarray(
                    W_proj[g * DLOC : (g + 1) * DLOC, :].astype(in_np)
                ),
                "mneg": mneg,
                "tri": tri.astype(in_np),
            }
        )
    return in_maps


def kernel(x, attention_mask, W_attn, b_attn, W_proj, b_proj, _res_hook=None):
    in_maps = make_in_maps(x, attention_mask, W_attn, b_attn, W_proj, b_proj)
    nc = _program()
    res = bass_utils.run_bass_kernel_spmd(nc, in_maps, core_ids=list(range(NCORES)))
    if _res_hook is not None:
        _res_hook(res)
    b_proj = np.asarray(b_proj, dtype=np.float32)
    y = np.zeros((B, T, C), dtype=np.float32)
    for c in range(NCORES):
        y[c // 4] += res.results[c]["out"]
    y += b_proj[None, None, :]
    return y

